# revision 21
# baseline (speedup 1.0000x reference)
"""Trainium2 Bass kernel for nn_DFlashDecoderLayer (dense transformer decoder layer:
self-attn + cross-attn + SwiGLU MLP, B=1, S=2048, H=1024, NH=16, HD=64, I=4096).

Sharding strategy (8 NeuronCores, SPMD):
  Sequence-sharded: core r owns query rows [256r, 256r+256).  Every matmul weight is
  used in full by every core, in fp8-e4m3 (scaled x64 host-side so sigma~1.3; the
  1/64 descale folds into existing epilogue ops).  The only cross-core dependency is
  full-sequence K/V for the two attention blocks; each core computes K/V for its own
  rows and a single fp8 AllGather per tensor shares them (512KB/rank -> 2MB out).

  On-chip layout is feature-major ("transposed"): activations live as [H_part, seq]
  so weight matrices ([in, out] row-major) serve directly as matmul lhsT tiles.
  All projection/MLP matmuls use fp8 DoubleRow (contraction 256 per MM via the
  [128, 2, M] packed access pattern).  Scores are computed transposed [k, q] in fp8
  (|scores| < ~3 so exp(scores) fits fp8 range without max-subtraction; probs fp8);
  the softmax denominator comes from a ones-column augmented onto V (M=65 matmul),
  and the 1/l division uses a K=1 ones outer-product matmul as the partition
  broadcast.  Residual path and softmax statistics stay fp32.
"""

import os
import sys

sys.path.insert(0, "/opt/trn_rl_repo")

import numpy as np
import ml_dtypes

import concourse.bass as bass
import concourse.mybir as mybir
import concourse.tile as tile

H = 1024      # hidden size
S = 2048      # sequence length
NH = 16       # heads
HD = 64       # head dim
I = 4096      # mlp intermediate
NC = 8        # cores
R = S // NC   # rows per core = 256
HT = H // 128  # hidden tiles = 8
KT = S // 128  # key tiles = 16
EPS = 1e-6
WS = 64.0     # host-side weight scale (keeps fp8 weights ~N(0, 1.3))
DS = 1.0 / WS  # descale folded into epilogues

F32 = mybir.dt.float32
BF16 = mybir.dt.bfloat16
FP8 = mybir.dt.float8e4
AF = mybir.ActivationFunctionType
ALU = mybir.AluOpType
DR = mybir.MatmulPerfMode.DoubleRow
BF16NP = ml_dtypes.bfloat16
FP8NP = ml_dtypes.float8_e4m3

_CACHED_MODULE = None


def _split_multi_waits(nc):
    """This env's walrus rejects >1 sem wait per instruction.
    Hoist extra waits onto preceding single-wait NoOps on the same engine."""
    limit = 1
    n_split = 0
    for f in nc.m.functions:
        for bb in f.blocks:
            new_insts = []
            for inst in bb.instructions:
                si = getattr(inst, "sync_info", None)
                if (si is not None and getattr(inst, "engine", None) is not None
                        and len(si.on_wait) > limit):
                    waits = list(si.on_wait)
                    hoist, keep = waits[:-limit], waits[-limit:]
                    for i, w in enumerate(hoist):
                        new_insts.append(
                            mybir.InstNoOp(
                                name=f"{inst.name}_waitsplit_{i}",
                                engine=inst.engine,
                                sync_info=mybir.SyncInfo(on_wait=[w], on_update=[]),
                                bass_nofuse=True,
                            )
                        )
                        n_split += 1
                    si.on_wait = keep
                new_insts.append(inst)
            bb.instructions = new_insts
    return n_split


def build_module():
    global _CACHED_MODULE
    if _CACHED_MODULE is not None:
        return _CACHED_MODULE

    nc = bass.Bass(num_devices=NC)

    # --- kernel I/O (per-core) ---
    xT = nc.declare_dram_parameter("xT", [H, R], F32, isOutput=False)
    ctxT = nc.declare_dram_parameter("ctxT", [H, R], FP8, isOutput=False)
    wnames = ["sa_wq", "sa_wk", "sa_wv", "sa_wo", "ca_wq", "ca_wk", "ca_wv", "ca_wo"]
    W = {n: nc.declare_dram_parameter(n, [H, H], FP8, isOutput=False) for n in wnames}
    W["w_gate"] = nc.declare_dram_parameter("w_gate", [H, I], BF16, isOutput=False)
    W["w_up"] = nc.declare_dram_parameter("w_up", [H, I], BF16, isOutput=False)
    W["w_down"] = nc.declare_dram_parameter("w_down", [I, H], FP8, isOutput=False)
    outT = nc.declare_dram_parameter("outT", [H, R], F32, isOutput=True)

    groups = [list(range(NC))]

    with tile.TileContext(nc) as tc:
        with (
            tc.tile_pool(name="p1", bufs=1) as p1,        # long-lived singles
            tc.tile_pool(name="p2", bufs=2) as p2,        # rotating pairs
            tc.tile_pool(name="resid", bufs=2) as presid, # xT / h1 / h2 fp32
            tc.tile_pool(name="wts", bufs=4) as pw,       # 1MB fp8 weight slabs
            tc.tile_pool(name="wtsb", bufs=2) as pwb,     # 2MB bf16 weight slabs (MLP)
            tc.tile_pool(name="psA", bufs=2, space="PSUM") as psA,
            tc.tile_pool(name="psB", bufs=2, space="PSUM") as psB,
            tc.tile_pool(name="pssc", bufs=2, space="PSUM") as pssc,  # 2-bank score tiles
            tc.tile_pool(name="dram", bufs=1, space="DRAM") as pdram,
        ):
            # --- warm up the collective stream first: the runtime's one-time
            # ~45us cross-core barrier attaches to the FIRST collective, so a
            # tiny dummy AllGather up front runs that barrier concurrently with
            # the prologue compute instead of gating the K1 gather. ---
            dumb_sb = p1.tile([1, 64], FP8, tag="dumb_sb")
            nc.vector.memset(dumb_sb[:], 1.0)
            dumb_in = pdram.tile([64], FP8, tag="dumb_in")
            dumb_out = pdram.tile([NC, 64], FP8, tag="dumb_out", addr_space="Shared")
            nc.sync.dma_start(dumb_in[:], dumb_sb[0, :])
            nc.gpsimd.collective_compute(
                "AllGather", mybir.AluOpType.bypass, replica_groups=groups,
                ins=[dumb_in[:]], outs=[dumb_out[:]])

            # --- constants ---
            inv_h = p1.tile([128, 1], F32, tag="inv_h")
            nc.vector.memset(inv_h[:], 1.0 / H)
            eps_c = p1.tile([1, 1], F32, tag="eps_c")
            nc.vector.memset(eps_c[:], EPS)
            # ones row for K=1 outer-product broadcasts
            ones_row = p1.tile([1, 128], F32, tag="ones_row")
            nc.vector.memset(ones_row[:], 1.0)
            # 8.0-row: the attnT = av/l product is scaled x8 so fp8 attnT stays
            # out of subnormal range (diffuse softmax -> |attn| ~ 0.014); the
            # o_proj epilogue descales by DS/8.
            ones_row8 = p1.tile([1, 128], F32, tag="ones_row8")
            nc.vector.memset(ones_row8[:], 8.0)
            # --- input loads ---
            xt_sb = presid.tile([128, HT, R], F32, tag="resid", name="xt_sb")
            nc.sync.dma_start(xt_sb[:], xT.rearrange("(t p) q -> p t q", p=128))
            ctx_sb = p1.tile([128, HT, R], FP8, tag="ctx_sb")
            nc.sync.dma_start(ctx_sb[:], ctxT.rearrange("(t p) q -> p t q", p=128))

            def load_w(dram_t, cols=None, rows=None, name="w", pool=None, dt=FP8):
                """Load a [1024, M<=1024] slab of a weight matrix as [128, 8, M]."""
                ap = dram_t.rearrange("(t p) m -> p t m", p=128)
                if rows is not None:  # row-chunk of a tall matrix (w_down)
                    ap = dram_t[rows[0]:rows[1], :].rearrange("(t p) m -> p t m", p=128)
                if cols is not None:
                    ap = ap[:, :, cols[0]:cols[1]]
                m = ap.shape[2]
                t = (pool or pw).tile([128, HT, 1024], dt, tag="w", name=name)
                nc.sync.dma_start(t[:, :, :m], ap)
                return t

            def rmsnorm(src_f32, dst_name, dt=FP8):
                """src [128, HT, R] f32 -> normalized [128, HT, R] (no weight)."""
                var = psA.tile([128, 512], F32, tag="psA", name=f"{dst_name}_var")
                for t in range(HT):
                    sq = p2.tile([128, R], F32, tag="sq", name=f"{dst_name}_sq{t}")
                    nc.vector.tensor_mul(sq[:], src_f32[:, t, :], src_f32[:, t, :])
                    nc.tensor.matmul(var[:1, :R], inv_h[:], sq[:],
                                     start=(t == 0), stop=(t == HT - 1))
                sd = p2.tile([1, R], F32, tag="sd", name=f"{dst_name}_sd")
                nc.scalar.activation(sd[:], var[:1, :R], AF.Sqrt, bias=eps_c[:])
                rstd = p2.tile([1, R], F32, tag="rstd", name=f"{dst_name}_rstd")
                nc.vector.reciprocal(rstd[:], sd[:])
                rb = psA.tile([128, 512], F32, tag="psA", name=f"{dst_name}_rb")
                nc.tensor.matmul(rb[:, :R], ones_row[:1, :], rstd[:1, :],
                                 start=True, stop=True)
                dst = p1.tile([128, HT, R], dt, tag="normed", name=dst_name)
                for t in range(HT):
                    nc.vector.tensor_mul(dst[:, t, :], src_f32[:, t, :], rb[:, :R])
                return dst

            def proj(w_sb, act_sb, dst_fp8, scale=DS):
                """dst[*, m, :] (fp8 [128, HT, R]) = scale * (W.T @ act)."""
                for m in range(HT):
                    ps = psA.tile([128, 512], F32, tag="psA", name=f"pj_{m}")
                    for t in range(HT // 2):
                        nc.tensor.matmul(ps[:, :R],
                                         w_sb[:, 2 * t:2 * t + 2, 128 * m:128 * (m + 1)],
                                         act_sb[:, 2 * t:2 * t + 2, :],
                                         start=(t == 0), stop=(t == HT // 2 - 1),
                                         perf_mode=DR)
                    nc.vector.tensor_scalar_mul(dst_fp8[:, m, :], ps[:, :R], scale)

            def proj_add(w_sb, act_sb, resid_f32, dst_f32, scale=DS):
                """dst (f32 [128, HT, R]) = resid + scale * (W.T @ act)."""
                for m in range(HT):
                    ps = psA.tile([128, 512], F32, tag="psA", name=f"pa_{m}")
                    for t in range(HT // 2):
                        nc.tensor.matmul(ps[:, :R],
                                         w_sb[:, 2 * t:2 * t + 2, 128 * m:128 * (m + 1)],
                                         act_sb[:, 2 * t:2 * t + 2, :],
                                         start=(t == 0), stop=(t == HT // 2 - 1),
                                         perf_mode=DR)
                    nc.vector.scalar_tensor_tensor(
                        dst_f32[:, m, :], ps[:, :R], scale, resid_f32[:, m, :],
                        ALU.mult, ALU.add)

            def kv_block(wk_sb, wv_sb, act_sb, blk):
                """Compute own-row K^T [1024, R] and V [R, 1024] (fp8, true scale),
                AllGather each across cores (K first so attention can start while
                the V gather is still in flight)."""
                k_in = pdram.tile([H * R], FP8, tag=f"kin{blk}")
                k_out = pdram.tile([NC, H * R], FP8, tag=f"kout{blk}",
                                   addr_space="Shared")
                v_in = pdram.tile([H * R], FP8, tag=f"vin{blk}")
                v_out = pdram.tile([NC, H * R], FP8, tag=f"vout{blk}",
                                   addr_space="Shared")
                k_view = k_in.rearrange("(t p q) -> p t q", t=HT, p=128, q=R)
                v_view = v_in.rearrange("(mt p d) -> p mt d", mt=2, p=128, d=1024)
                for m in range(HT):
                    ps = psA.tile([128, 512], F32, tag="psA", name=f"k{blk}_{m}")
                    for t in range(HT // 2):
                        nc.tensor.matmul(ps[:, :R],
                                         wk_sb[:, 2 * t:2 * t + 2, 128 * m:128 * (m + 1)],
                                         act_sb[:, 2 * t:2 * t + 2, :],
                                         start=(t == 0), stop=(t == HT // 2 - 1),
                                         perf_mode=DR)
                    stg = p2.tile([128, 512], FP8, tag="stg", bufs=3, name=f"ks{blk}_{m}")
                    nc.vector.tensor_scalar_mul(stg[:, :R], ps[:, :R], DS)
                    nc.sync.dma_start(k_view[:, m, :], stg[:, :R])
                nc.gpsimd.collective_compute(
                    "AllGather", mybir.AluOpType.bypass, replica_groups=groups,
                    ins=[k_in[:]], outs=[k_out[:]])
                # V row-major [256 seq rows, 1024]: lhsT = act (stationary),
                # rhs = wv columns; DR needs rhs free <= 512 so 256-col chunks.
                for mt in range(2):
                    for nch in range(2):
                        ps = psA.tile([128, 512], F32, tag="psA", name=f"v{blk}_{mt}_{nch}")
                        for half in range(2):
                            col0 = 512 * nch + 256 * half
                            for t in range(HT // 2):
                                nc.tensor.matmul(
                                    ps[:, 256 * half:256 * (half + 1)],
                                    act_sb[:, 2 * t:2 * t + 2, 128 * mt:128 * (mt + 1)],
                                    wv_sb[:, 2 * t:2 * t + 2, col0:col0 + 256],
                                    start=(t == 0), stop=(t == HT // 2 - 1),
                                    perf_mode=DR)
                        stg = p2.tile([128, 512], FP8, tag="stg", bufs=3,
                                      name=f"vs{blk}_{mt}_{nch}")
                        nc.vector.tensor_scalar_mul(stg[:], ps[:], DS)
                        nc.sync.dma_start(v_view[:, mt, 512 * nch:512 * (nch + 1)], stg[:])
                nc.gpsimd.collective_compute(
                    "AllGather", mybir.AluOpType.bypass, replica_groups=groups,
                    ins=[v_in[:]], outs=[v_out[:]])
                return k_out, v_out

            def attention(q_sb, k_out, v_out, blk):
                """q_sb [128, HT, R] fp8 (feature-major, all heads, values x1/8),
                k_out/v_out from kv_block. Returns attnT [128, HT, R] fp8."""
                vsb = p1.tile([128, KT, NH, HD + 1], FP8, tag="vsb", name=f"vsb{blk}")
                attnT = p1.tile([128, HT, R], FP8, tag="attnT", name=f"attnT{blk}")
                # V table loads go on the gpsimd (SWDGE) queue: they wait on the
                # V AllGather, and on the sync queue they'd head-of-line block the
                # K-tile loads that later head-pairs' scores need.
                for kt in range(KT):
                    r, mt = kt // 2, kt % 2
                    src = v_out[r].rearrange(
                        "(mt p hd d) -> p mt hd d", mt=2, p=128, hd=NH, d=HD)
                    nc.gpsimd.dma_start(vsb[:, kt, :, 0:HD], src[:, mt, :, :])
                nc.vector.memset(vsb[:, :, :, HD:HD + 1], 1.0)
                for dt in range(HT):  # head pair dt = heads 2dt, 2dt+1
                    kf = p2.tile([128, NC, R], FP8, tag="kf", bufs=4, name=f"kf{blk}_{dt}")
                    for r in range(NC):
                        nc.sync.dma_start(
                            kf[:, r, :],
                            k_out[r].rearrange("(t p q) -> p t q", t=HT, p=128, q=R)[:, dt, :])
                    # Both heads' score matmuls are emitted ADJACENTLY so the
                    # (0,0)/(64,0) tile_position row-packing runs them
                    # concurrently on the PE array.  Scores for 4 k-tiles land in
                    # a 2-bank psum tile -> one exp covers N=1024.
                    pts = [p2.tile([128, KT, R], FP8, tag="pt", bufs=3,
                                   name=f"pt{blk}_{2 * dt + hh}") for hh in range(2)]
                    for c in range(KT // 4):
                        spss = [pssc.tile([128, 4, 256], F32, tag="pssc",
                                          name=f"s{blk}_{2 * dt + hh}_{c}") for hh in range(2)]
                        for i in range(4):
                            kt = 4 * c + i
                            r2, mt2 = kt // 2, kt % 2
                            for hh in range(2):
                                off = HD * hh
                                nc.tensor.matmul(
                                    spss[hh][:, i, :],
                                    kf[off:off + HD, r2, 128 * mt2:128 * (mt2 + 1)],
                                    q_sb[off:off + HD, dt, :],
                                    start=True, stop=True, tile_position=(off, 0))
                        for hh in range(2):
                            # q was staged at x1/4: exp(score/4 * 4) via scale
                            nc.scalar.activation(
                                pts[hh][:, 4 * c:4 * c + 4, :],
                                spss[hh][:, :, :], AF.Exp, scale=4.0)
                    # AV after all scores of the head: the PE stream is in-order,
                    # so an early AV waiting on the V gather would head-of-line
                    # block the remaining score matmuls.
                    for hh in range(2):
                        h = 2 * dt + hh
                        off = HD * hh
                        pt = pts[hh]
                        avps = psB.tile([128, 512], F32, tag="psB", name=f"av{blk}_{h}")
                        for k2 in range(KT // 2):  # DR over kpos pairs (probs fp8)
                            nc.tensor.matmul(avps[:HD + 1, :R],
                                             vsb[:, 2 * k2:2 * k2 + 2, h, :],
                                             pt[:, 2 * k2:2 * k2 + 2, :],
                                             start=(k2 == 0), stop=(k2 == KT // 2 - 1),
                                             perf_mode=DR)
                        rl = p2.tile([1, R], F32, tag="rl", name=f"rl{blk}_{h}")
                        nc.vector.reciprocal(rl[:], avps[HD:HD + 1, :R])
                        rlb = psB.tile([128, 512], F32, tag="psB", name=f"rlb{blk}_{h}")
                        nc.tensor.matmul(rlb[:HD, :R], ones_row8[:1, :HD], rl[:1, :],
                                         start=True, stop=True)
                        av_sb = p2.tile([HD, R], F32, tag="av_sb", name=f"avs{blk}_{h}")
                        nc.vector.tensor_copy(av_sb[:], avps[0:HD, :R])
                        nc.vector.tensor_mul(attnT[off:off + HD, dt, :],
                                             av_sb[:], rlb[:HD, :R])
                return attnT

            # ---------------- self-attention block ----------------
            xn = rmsnorm(xt_sb, "xn")
            wk_sb = load_w(W["sa_wk"], name="sa_wk_sb")
            wv_sb = load_w(W["sa_wv"], name="sa_wv_sb")
            k1, v1 = kv_block(wk_sb, wv_sb, xn, 0)

            # cross-attn K/V depend only on raw context: compute + AG them early
            # so both gathers overlap the self-attention epilogue projections.
            wk2_sb = load_w(W["ca_wk"], name="ca_wk_sb")
            wv2_sb = load_w(W["ca_wv"], name="ca_wv_sb")
            k2, v2 = kv_block(wk2_sb, wv2_sb, ctx_sb, 1)

            wq_sb = load_w(W["sa_wq"], name="sa_wq_sb")
            qT = p1.tile([128, HT, R], FP8, tag="qt", name="qT")
            proj(wq_sb, xn, qT, scale=DS / 4.0)  # q staged at x1/4 (fp8 range)

            attnT = attention(qT, k1, v1, 0)
            wo_sb = load_w(W["sa_wo"], name="sa_wo_sb")
            h1 = presid.tile([128, HT, R], F32, tag="resid", name="h1")
            proj_add(wo_sb, attnT, xt_sb, h1, scale=DS / 8.0)  # attnT is x8

            # ---------------- cross-attention block ----------------
            hn = rmsnorm(h1, "hn")
            wq2_sb = load_w(W["ca_wq"], name="ca_wq_sb")
            qT2 = p1.tile([128, HT, R], FP8, tag="qt", name="qT2")
            proj(wq2_sb, hn, qT2, scale=DS / 4.0)

            attnT2 = attention(qT2, k2, v2, 1)
            wo2_sb = load_w(W["ca_wo"], name="ca_wo_sb")
            h2 = presid.tile([128, HT, R], F32, tag="resid", name="h2")
            proj_add(wo2_sb, attnT2, h1, h2, scale=DS / 8.0)

            # ---------------- MLP block ----------------
            # NOTE: start=True clears has_written for the WHOLE psum bank, so each
            # accumulation group must own its bank exclusively for its entire
            # lifetime.  Phase A computes all 32 act subtiles into SBUF; phase B
            # runs one contiguous accumulation per output tile.
            # The MLP path carries most of the output magnitude (mlp_out sigma
            # ~0.38 vs attn-block outs ~0.01), so fp8 gate/up/act there costs
            # ~1.8% L2 error.  gate/up weights and the activation tensor run in
            # bf16; w_down stays fp8 as the lhsT of a mixed fp8 x bf16 matmul.
            hn2b = rmsnorm(h2, "hn2", dt=BF16)
            NCHUNK = 4  # I-chunks of 1024
            act_full = p1.tile([128, I // 128, R], BF16, tag="act_full")  # 2MB
            wds = []
            for c in range(NCHUNK):
                wg_sb = load_w(W["w_gate"], cols=(1024 * c, 1024 * (c + 1)), name=f"wg{c}",
                               pool=pwb, dt=BF16)
                wu_sb = load_w(W["w_up"], cols=(1024 * c, 1024 * (c + 1)), name=f"wu{c}",
                               pool=pwb, dt=BF16)
                for mi in range(8):
                    gps = psA.tile([128, 512], F32, tag="psA", name=f"g{c}_{mi}")
                    for t in range(HT):
                        nc.tensor.matmul(gps[:, :R],
                                         wg_sb[:, t, 128 * mi:128 * (mi + 1)],
                                         hn2b[:, t, :],
                                         start=(t == 0), stop=(t == HT - 1))
                    ups = psA.tile([128, 512], F32, tag="psA", name=f"u{c}_{mi}")
                    for t in range(HT):
                        nc.tensor.matmul(ups[:, :R],
                                         wu_sb[:, t, 128 * mi:128 * (mi + 1)],
                                         hn2b[:, t, :],
                                         start=(t == 0), stop=(t == HT - 1))
                    gsil = p2.tile([128, R], BF16, tag="gsil", name=f"gs{c}_{mi}")
                    nc.scalar.activation(gsil[:], gps[:, :R], AF.Silu, scale=DS)
                    # act = (up * DS) * silu(gate * DS), bf16 out
                    nc.vector.scalar_tensor_tensor(
                        act_full[:, 8 * c + mi, :], ups[:, :R], DS, gsil[:],
                        ALU.mult, ALU.mult)
            for c in range(NCHUNK):
                wds.append(load_w(W["w_down"], rows=(1024 * c, 1024 * (c + 1)), name=f"wd{c}"))
            out_sb = p1.tile([128, HT, R], F32, tag="out_sb")
            for m in range(HT):
                dps = psB.tile([128, 512], F32, tag="psB", name=f"dp{m}")
                for s in range(I // 128):
                    wd = wds[s // 8]
                    nc.tensor.matmul(dps[:, :R],
                                     wd[:, s % 8, 128 * m:128 * (m + 1)],
                                     act_full[:, s, :],
                                     start=(s == 0), stop=(s == I // 128 - 1))
                nc.vector.scalar_tensor_tensor(
                    out_sb[:, m, :], dps[:, :R], DS, h2[:, m, :], ALU.mult, ALU.add)
            nc.sync.dma_start(outT.rearrange("(t p) q -> p t q", p=128), out_sb[:])

    _split_multi_waits(nc)
    _CACHED_MODULE = nc
    return nc


def prep_in_maps(hidden_states, context, sa_norm_w, sa_wq, sa_wk, sa_wv, sa_wo,
                 ca_norm_w, ca_wq, ca_wk, ca_wv, ca_wo,
                 mlp_norm_w, w_gate, w_up, w_down):
    f32 = np.float32
    x = np.asarray(hidden_states, f32).reshape(S, H)
    ctx = np.asarray(context, f32).reshape(S, H)
    xT_full = np.ascontiguousarray(x.T)                      # [H, S] f32
    ctxT_full = np.ascontiguousarray(ctx.T).astype(FP8NP)    # [H, S] fp8

    def f8(a):
        return np.ascontiguousarray(
            np.clip(np.asarray(a, f32) * WS, -240.0, 240.0)).astype(FP8NP)

    def bf(a):
        return np.ascontiguousarray(np.asarray(a, f32)).astype(BF16NP)

    sa_w = np.asarray(sa_norm_w, f32)[:, None]
    ca_w = np.asarray(ca_norm_w, f32)[:, None]
    mlp_w = np.asarray(mlp_norm_w, f32)[:, None]
    scale = HD ** -0.5
    shared = {
        "sa_wq": f8(sa_w * np.asarray(sa_wq, f32) * scale),
        "sa_wk": f8(sa_w * np.asarray(sa_wk, f32)),
        "sa_wv": f8(sa_w * np.asarray(sa_wv, f32)),
        "sa_wo": f8(sa_wo),
        "ca_wq": f8(ca_w * np.asarray(ca_wq, f32) * scale),
        "ca_wk": f8(ca_wk),
        "ca_wv": f8(ca_wv),
        "ca_wo": f8(ca_wo),
        "w_gate": bf(mlp_w * np.asarray(w_gate, f32) * WS),
        "w_up": bf(mlp_w * np.asarray(w_up, f32) * WS),
        "w_down": f8(w_down),
    }
    in_maps = []
    for r in range(NC):
        m = dict(shared)
        m["xT"] = np.ascontiguousarray(xT_full[:, r * R:(r + 1) * R])
        m["ctxT"] = np.ascontiguousarray(ctxT_full[:, r * R:(r + 1) * R])
        in_maps.append(m)
    return in_maps


def run_spmd(in_maps, **kwargs):
    from concourse.bass_utils import run_bass_kernel_spmd
    nc = build_module()
    return run_bass_kernel_spmd(nc, in_maps, core_ids=list(range(NC)), **kwargs)


def kernel(**inputs):
    in_maps = prep_in_maps(**inputs)
    res = run_spmd(in_maps)
    out = np.empty((1, S, H), np.float32)
    for r in range(NC):
        out[0, r * R:(r + 1) * R, :] = res.results[r]["outT"].T
    return out


# revision 27
# speedup vs baseline: 1.0672x; 1.0672x over previous
"""Trainium2 Bass kernel for nn_DFlashDecoderLayer (dense transformer decoder layer:
self-attn + cross-attn + SwiGLU MLP, B=1, S=2048, H=1024, NH=16, HD=64, I=4096).

Sharding strategy (8 NeuronCores, SPMD):
  Sequence-sharded: core r owns query rows [256r, 256r+256).  Every matmul weight is
  used in full by every core, in fp8-e4m3 (scaled x64 host-side so sigma~1.3; the
  1/64 descale folds into existing epilogue ops).  The only cross-core dependency is
  full-sequence K/V for the two attention blocks; each core computes K/V for its own
  rows and a single fp8 AllGather per tensor shares them (512KB/rank -> 2MB out).

  On-chip layout is feature-major ("transposed"): activations live as [H_part, seq]
  so weight matrices ([in, out] row-major) serve directly as matmul lhsT tiles.
  All projection/MLP matmuls use fp8 DoubleRow (contraction 256 per MM via the
  [128, 2, M] packed access pattern).  Scores are computed transposed [k, q] in fp8
  (|scores| < ~3 so exp(scores) fits fp8 range without max-subtraction; probs fp8);
  the softmax denominator comes from a ones-column augmented onto V (M=65 matmul),
  and the 1/l division uses a K=1 ones outer-product matmul as the partition
  broadcast.  Residual path and softmax statistics stay fp32.
"""

import os
import sys

sys.path.insert(0, "/opt/trn_rl_repo")

import numpy as np
import ml_dtypes

import concourse.bass as bass
import concourse.mybir as mybir
import concourse.tile as tile

H = 1024      # hidden size
S = 2048      # sequence length
NH = 16       # heads
HD = 64       # head dim
I = 4096      # mlp intermediate
NC = 8        # cores
R = S // NC   # rows per core = 256
HT = H // 128  # hidden tiles = 8
KT = S // 128  # key tiles = 16
EPS = 1e-6
WS = 64.0     # host-side weight scale (keeps fp8 weights ~N(0, 1.3))
DS = 1.0 / WS  # descale folded into epilogues

F32 = mybir.dt.float32
BF16 = mybir.dt.bfloat16
FP8 = mybir.dt.float8e4
AF = mybir.ActivationFunctionType
ALU = mybir.AluOpType
DR = mybir.MatmulPerfMode.DoubleRow
BF16NP = ml_dtypes.bfloat16
FP8NP = ml_dtypes.float8_e4m3

_CACHED_MODULE = None


def _split_multi_waits(nc):
    """This env's walrus rejects >1 sem wait per instruction.
    Hoist extra waits onto preceding single-wait NoOps on the same engine."""
    limit = 1
    n_split = 0
    for f in nc.m.functions:
        for bb in f.blocks:
            new_insts = []
            for inst in bb.instructions:
                si = getattr(inst, "sync_info", None)
                if (si is not None and getattr(inst, "engine", None) is not None
                        and len(si.on_wait) > limit):
                    waits = list(si.on_wait)
                    hoist, keep = waits[:-limit], waits[-limit:]
                    for i, w in enumerate(hoist):
                        new_insts.append(
                            mybir.InstNoOp(
                                name=f"{inst.name}_waitsplit_{i}",
                                engine=inst.engine,
                                sync_info=mybir.SyncInfo(on_wait=[w], on_update=[]),
                                bass_nofuse=True,
                            )
                        )
                        n_split += 1
                    si.on_wait = keep
                new_insts.append(inst)
            bb.instructions = new_insts
    return n_split


def build_module():
    global _CACHED_MODULE
    if _CACHED_MODULE is not None:
        return _CACHED_MODULE

    nc = bass.Bass(num_devices=NC)

    # --- kernel I/O (per-core) ---
    xT = nc.declare_dram_parameter("xT", [H, R], F32, isOutput=False)
    ctxT = nc.declare_dram_parameter("ctxT", [H, R], FP8, isOutput=False)
    wnames = ["sa_wq", "sa_wk", "sa_wv", "sa_wo", "ca_wq", "ca_wk", "ca_wv", "ca_wo"]
    W = {n: nc.declare_dram_parameter(n, [H, H], FP8, isOutput=False) for n in wnames}
    W["w_gate"] = nc.declare_dram_parameter("w_gate", [H, I], BF16, isOutput=False)
    W["w_up"] = nc.declare_dram_parameter("w_up", [H, I], BF16, isOutput=False)
    W["w_down"] = nc.declare_dram_parameter("w_down", [I, H], FP8, isOutput=False)
    outT = nc.declare_dram_parameter("outT", [H, R], F32, isOutput=True)

    groups = [list(range(NC))]

    with tile.TileContext(nc) as tc:
        with (
            tc.tile_pool(name="p1", bufs=1) as p1,        # long-lived singles
            tc.tile_pool(name="p2", bufs=2) as p2,        # rotating pairs
            tc.tile_pool(name="resid", bufs=2) as presid, # xT / h1 / h2 fp32
            tc.tile_pool(name="wts", bufs=4) as pw,       # 1MB fp8 weight slabs
            tc.tile_pool(name="wtsb", bufs=2) as pwb,     # 2MB bf16 weight slabs (MLP)
            tc.tile_pool(name="psA", bufs=2, space="PSUM") as psA,
            tc.tile_pool(name="psB", bufs=2, space="PSUM") as psB,
            tc.tile_pool(name="pssc", bufs=2, space="PSUM") as pssc,  # 2-bank score tiles
            tc.tile_pool(name="dram", bufs=1, space="DRAM") as pdram,
        ):
            # --- constants ---
            inv_h = p1.tile([128, 1], F32, tag="inv_h")
            nc.vector.memset(inv_h[:], 1.0 / H)
            eps_c = p1.tile([1, 1], F32, tag="eps_c")
            nc.vector.memset(eps_c[:], EPS)
            # ones row for K=1 outer-product broadcasts
            ones_row = p1.tile([1, 128], F32, tag="ones_row")
            nc.vector.memset(ones_row[:], 1.0)
            # 8.0-row: the attnT = av/l product is scaled x8 so fp8 attnT stays
            # out of subnormal range (diffuse softmax -> |attn| ~ 0.014); the
            # o_proj epilogue descales by DS/8.
            ones_row8 = p1.tile([1, 128], F32, tag="ones_row8")
            nc.vector.memset(ones_row8[:], 8.0)
            # --- input loads ---
            xt_sb = presid.tile([128, HT, R], F32, tag="resid", name="xt_sb")
            nc.sync.dma_start(xt_sb[:], xT.rearrange("(t p) q -> p t q", p=128))
            ctx_sb = p1.tile([128, HT, R], FP8, tag="ctx_sb")
            nc.sync.dma_start(ctx_sb[:], ctxT.rearrange("(t p) q -> p t q", p=128))

            def load_w(dram_t, cols=None, rows=None, name="w", pool=None, dt=FP8):
                """Load a [1024, M<=1024] slab of a weight matrix as [128, 8, M]."""
                ap = dram_t.rearrange("(t p) m -> p t m", p=128)
                if rows is not None:  # row-chunk of a tall matrix (w_down)
                    ap = dram_t[rows[0]:rows[1], :].rearrange("(t p) m -> p t m", p=128)
                if cols is not None:
                    ap = ap[:, :, cols[0]:cols[1]]
                m = ap.shape[2]
                t = (pool or pw).tile([128, HT, 1024], dt, tag="w", name=name)
                nc.sync.dma_start(t[:, :, :m], ap)
                return t

            def rmsnorm(src_f32, dst_name, dt=FP8):
                """src [128, HT, R] f32 -> normalized [128, HT, R] (no weight)."""
                var = psA.tile([128, 512], F32, tag="psA", name=f"{dst_name}_var")
                for t in range(HT):
                    sq = p2.tile([128, R], F32, tag="sq", name=f"{dst_name}_sq{t}")
                    nc.vector.tensor_mul(sq[:], src_f32[:, t, :], src_f32[:, t, :])
                    nc.tensor.matmul(var[:1, :R], inv_h[:], sq[:],
                                     start=(t == 0), stop=(t == HT - 1))
                sd = p2.tile([1, R], F32, tag="sd", name=f"{dst_name}_sd")
                nc.scalar.activation(sd[:], var[:1, :R], AF.Sqrt, bias=eps_c[:])
                rstd = p2.tile([1, R], F32, tag="rstd", name=f"{dst_name}_rstd")
                nc.vector.reciprocal(rstd[:], sd[:])
                rb = psA.tile([128, 512], F32, tag="psA", name=f"{dst_name}_rb")
                nc.tensor.matmul(rb[:, :R], ones_row[:1, :], rstd[:1, :],
                                 start=True, stop=True)
                dst = p1.tile([128, HT, R], dt, tag="normed", name=dst_name)
                for t in range(HT):
                    nc.vector.tensor_mul(dst[:, t, :], src_f32[:, t, :], rb[:, :R])
                return dst

            def proj(w_sb, act_sb, dst_fp8, scale=DS):
                """dst[*, m, :] (fp8 [128, HT, R]) = scale * (W.T @ act)."""
                for m in range(HT):
                    ps = psA.tile([128, 512], F32, tag="psA", name=f"pj_{m}")
                    for t in range(HT // 2):
                        nc.tensor.matmul(ps[:, :R],
                                         w_sb[:, 2 * t:2 * t + 2, 128 * m:128 * (m + 1)],
                                         act_sb[:, 2 * t:2 * t + 2, :],
                                         start=(t == 0), stop=(t == HT // 2 - 1),
                                         perf_mode=DR)
                    nc.vector.tensor_scalar_mul(dst_fp8[:, m, :], ps[:, :R], scale)

            def proj_add(w_sb, act_sb, resid_f32, dst_f32, scale=DS):
                """dst (f32 [128, HT, R]) = resid + scale * (W.T @ act)."""
                for m in range(HT):
                    ps = psA.tile([128, 512], F32, tag="psA", name=f"pa_{m}")
                    for t in range(HT // 2):
                        nc.tensor.matmul(ps[:, :R],
                                         w_sb[:, 2 * t:2 * t + 2, 128 * m:128 * (m + 1)],
                                         act_sb[:, 2 * t:2 * t + 2, :],
                                         start=(t == 0), stop=(t == HT // 2 - 1),
                                         perf_mode=DR)
                    nc.vector.scalar_tensor_tensor(
                        dst_f32[:, m, :], ps[:, :R], scale, resid_f32[:, m, :],
                        ALU.mult, ALU.add)

            def kv_block(wk_sb, wv_sb, act_sb, blk, merge=False):
                """Compute own-row K^T [1024, R] and V [R, 1024] (fp8, true scale),
                AllGather across cores.  merge=False: two AGs, K first, so
                attention can start while the V gather is in flight.  merge=True
                (cross-attn: both needed late): one combined AG, fewer cc ops."""
                if merge:
                    kv_in = pdram.tile([2 * H * R], FP8, tag=f"kvin{blk}")
                    kv_out = pdram.tile([NC, 2 * H * R], FP8, tag=f"kvout{blk}",
                                        addr_space="Shared")
                    k_in, v_in = kv_in[0:H * R], kv_in[H * R:2 * H * R]
                    kfn = lambda r: kv_out[r, 0:H * R]
                    vfn = lambda r: kv_out[r, H * R:2 * H * R]
                else:
                    k_in_t = pdram.tile([H * R], FP8, tag=f"kin{blk}")
                    k_out = pdram.tile([NC, H * R], FP8, tag=f"kout{blk}",
                                       addr_space="Shared")
                    v_in_t = pdram.tile([H * R], FP8, tag=f"vin{blk}")
                    v_out = pdram.tile([NC, H * R], FP8, tag=f"vout{blk}",
                                       addr_space="Shared")
                    k_in, v_in = k_in_t[:], v_in_t[:]
                    kfn = lambda r: k_out[r]
                    vfn = lambda r: v_out[r]
                k_view = k_in.rearrange("(t p q) -> p t q", t=HT, p=128, q=R)
                v_view = v_in.rearrange("(mt p d) -> p mt d", mt=2, p=128, d=1024)
                for m in range(HT):
                    ps = psA.tile([128, 512], F32, tag="psA", name=f"k{blk}_{m}")
                    for t in range(HT // 2):
                        nc.tensor.matmul(ps[:, :R],
                                         wk_sb[:, 2 * t:2 * t + 2, 128 * m:128 * (m + 1)],
                                         act_sb[:, 2 * t:2 * t + 2, :],
                                         start=(t == 0), stop=(t == HT // 2 - 1),
                                         perf_mode=DR)
                    stg = p2.tile([128, 512], FP8, tag="stg", bufs=3, name=f"ks{blk}_{m}")
                    nc.vector.tensor_scalar_mul(stg[:, :R], ps[:, :R], DS)
                    nc.sync.dma_start(k_view[:, m, :], stg[:, :R])
                if not merge:
                    nc.gpsimd.collective_compute(
                        "AllGather", mybir.AluOpType.bypass, replica_groups=groups,
                        ins=[k_in], outs=[k_out[:]])
                # V row-major [256 seq rows, 1024]: lhsT = act (stationary),
                # rhs = wv columns; DR needs rhs free <= 512 so 256-col chunks.
                for mt in range(2):
                    for nch in range(2):
                        ps = psA.tile([128, 512], F32, tag="psA", name=f"v{blk}_{mt}_{nch}")
                        for half in range(2):
                            col0 = 512 * nch + 256 * half
                            for t in range(HT // 2):
                                nc.tensor.matmul(
                                    ps[:, 256 * half:256 * (half + 1)],
                                    act_sb[:, 2 * t:2 * t + 2, 128 * mt:128 * (mt + 1)],
                                    wv_sb[:, 2 * t:2 * t + 2, col0:col0 + 256],
                                    start=(t == 0), stop=(t == HT // 2 - 1),
                                    perf_mode=DR)
                        stg = p2.tile([128, 512], FP8, tag="stg", bufs=3,
                                      name=f"vs{blk}_{mt}_{nch}")
                        nc.vector.tensor_scalar_mul(stg[:], ps[:], DS)
                        nc.sync.dma_start(v_view[:, mt, 512 * nch:512 * (nch + 1)], stg[:])
                if merge:
                    nc.gpsimd.collective_compute(
                        "AllGather", mybir.AluOpType.bypass, replica_groups=groups,
                        ins=[kv_in[:]], outs=[kv_out[:]])
                else:
                    nc.gpsimd.collective_compute(
                        "AllGather", mybir.AluOpType.bypass, replica_groups=groups,
                        ins=[v_in], outs=[v_out[:]])
                return kfn, vfn

            def attention(q_sb, kfn, vfn, blk):
                """q_sb [128, HT, R] fp8 (feature-major, all heads, values x1/4),
                kfn/vfn rank->AP accessors from kv_block.
                Returns attnT [128, HT, R] fp8 (values x8)."""
                vsb = p1.tile([128, KT, NH, HD + 1], FP8, tag="vsb", name=f"vsb{blk}")
                attnT = p1.tile([128, HT, R], FP8, tag="attnT", name=f"attnT{blk}")
                # V table loads go on the gpsimd (SWDGE) queue: they wait on the
                # V AllGather, and on the sync queue they'd head-of-line block the
                # K-tile loads that later head-pairs' scores need.
                for kt in range(KT):
                    r, mt = kt // 2, kt % 2
                    src = vfn(r).rearrange(
                        "(mt p hd d) -> p mt hd d", mt=2, p=128, hd=NH, d=HD)
                    nc.gpsimd.dma_start(vsb[:, kt, :, 0:HD], src[:, mt, :, :])
                nc.vector.memset(vsb[:, :, :, HD:HD + 1], 1.0)

                def scores(dt):
                    """Score matmuls + exp for head pair dt -> pts (probs fp8).
                    Both heads' matmuls are emitted ADJACENTLY so the (0,0)/(64,0)
                    tile_position row-packing runs them concurrently.  Scores for
                    4 k-tiles land in a 2-bank psum tile -> one exp per N=1024."""
                    kf = p2.tile([128, NC, R], FP8, tag="kf", bufs=4, name=f"kf{blk}_{dt}")
                    for r in range(NC):
                        nc.sync.dma_start(
                            kf[:, r, :],
                            kfn(r).rearrange("(t p q) -> p t q", t=HT, p=128, q=R)[:, dt, :])
                    pts = [p2.tile([128, KT, R], FP8, tag="pt", bufs=10,
                                   name=f"pt{blk}_{2 * dt + hh}") for hh in range(2)]
                    for c in range(KT // 4):
                        spss = [pssc.tile([128, 4, 256], F32, tag="pssc",
                                          name=f"s{blk}_{2 * dt + hh}_{c}") for hh in range(2)]
                        for i in range(4):
                            kt = 4 * c + i
                            r2, mt2 = kt // 2, kt % 2
                            for hh in range(2):
                                off = HD * hh
                                nc.tensor.matmul(
                                    spss[hh][:, i, :],
                                    kf[off:off + HD, r2, 128 * mt2:128 * (mt2 + 1)],
                                    q_sb[off:off + HD, dt, :],
                                    start=True, stop=True, tile_position=(off, 0))
                        for hh in range(2):
                            # q was staged at x1/4: exp(score/4 * 4) via scale
                            nc.scalar.activation(
                                pts[hh][:, 4 * c:4 * c + 4, :],
                                spss[hh][:, :, :], AF.Exp, scale=4.0)
                    return pts

                def av(dt, pts):
                    """AV + 1/l epilogue for head pair dt (DR over kpos pairs)."""
                    for hh in range(2):
                        h = 2 * dt + hh
                        off = HD * hh
                        pt = pts[hh]
                        avps = psB.tile([128, 512], F32, tag="psB", name=f"av{blk}_{h}")
                        for k2 in range(KT // 2):
                            nc.tensor.matmul(avps[:HD + 1, :R],
                                             vsb[:, 2 * k2:2 * k2 + 2, h, :],
                                             pt[:, 2 * k2:2 * k2 + 2, :],
                                             start=(k2 == 0), stop=(k2 == KT // 2 - 1),
                                             perf_mode=DR)
                        rl = p2.tile([1, R], F32, tag="rl", name=f"rl{blk}_{h}")
                        nc.vector.reciprocal(rl[:], avps[HD:HD + 1, :R])
                        rlb = psB.tile([128, 512], F32, tag="psB", name=f"rlb{blk}_{h}")
                        nc.tensor.matmul(rlb[:HD, :R], ones_row8[:1, :HD], rl[:1, :],
                                         start=True, stop=True)
                        av_sb = p2.tile([HD, R], F32, tag="av_sb", name=f"avs{blk}_{h}")
                        nc.vector.tensor_copy(av_sb[:], avps[0:HD, :R])
                        nc.vector.tensor_mul(attnT[off:off + HD, dt, :],
                                             av_sb[:], rlb[:HD, :R])

                # Software pipeline with AV deferred LOOKAHEAD pairs: the PE
                # stream is in-order, so the first AVs (waiting on the V gather)
                # would otherwise head-of-line block later pairs' score matmuls.
                LOOKAHEAD = 3
                pending = []
                for dt in range(HT):
                    pending.append((dt, scores(dt)))
                    if len(pending) > LOOKAHEAD:
                        d0, p0 = pending.pop(0)
                        av(d0, p0)
                for d0, p0 in pending:
                    av(d0, p0)
                return attnT

            # ---------------- self-attention block ----------------
            xn = rmsnorm(xt_sb, "xn")
            wk_sb = load_w(W["sa_wk"], name="sa_wk_sb")
            wv_sb = load_w(W["sa_wv"], name="sa_wv_sb")
            k1, v1 = kv_block(wk_sb, wv_sb, xn, 0)

            # cross-attn K/V depend only on raw context: compute + AG them early
            # so both gathers overlap the self-attention epilogue projections.
            wk2_sb = load_w(W["ca_wk"], name="ca_wk_sb")
            wv2_sb = load_w(W["ca_wv"], name="ca_wv_sb")
            k2, v2 = kv_block(wk2_sb, wv2_sb, ctx_sb, 1, merge=True)

            wq_sb = load_w(W["sa_wq"], name="sa_wq_sb")
            qT = p1.tile([128, HT, R], FP8, tag="qt", name="qT")
            proj(wq_sb, xn, qT, scale=DS / 4.0)  # q staged at x1/4 (fp8 range)

            attnT = attention(qT, k1, v1, 0)
            wo_sb = load_w(W["sa_wo"], name="sa_wo_sb")
            h1 = presid.tile([128, HT, R], F32, tag="resid", name="h1")
            proj_add(wo_sb, attnT, xt_sb, h1, scale=DS / 8.0)  # attnT is x8

            # ---------------- cross-attention block ----------------
            hn = rmsnorm(h1, "hn")
            wq2_sb = load_w(W["ca_wq"], name="ca_wq_sb")
            qT2 = p1.tile([128, HT, R], FP8, tag="qt", name="qT2")
            proj(wq2_sb, hn, qT2, scale=DS / 4.0)

            attnT2 = attention(qT2, k2, v2, 1)
            wo2_sb = load_w(W["ca_wo"], name="ca_wo_sb")
            h2 = presid.tile([128, HT, R], F32, tag="resid", name="h2")
            proj_add(wo2_sb, attnT2, h1, h2, scale=DS / 8.0)

            # ---------------- MLP block ----------------
            # NOTE: start=True clears has_written for the WHOLE psum bank, so each
            # accumulation group must own its bank exclusively for its entire
            # lifetime.  Phase A computes all 32 act subtiles into SBUF; phase B
            # runs one contiguous accumulation per output tile.
            # The MLP path carries most of the output magnitude (mlp_out sigma
            # ~0.38 vs attn-block outs ~0.01), so fp8 gate/up/act there costs
            # ~1.8% L2 error.  gate/up weights and the activation tensor run in
            # bf16; w_down stays fp8 as the lhsT of a mixed fp8 x bf16 matmul.
            hn2b = rmsnorm(h2, "hn2", dt=BF16)
            NCHUNK = 4  # I-chunks of 1024
            act_full = p1.tile([128, I // 128, R], BF16, tag="act_full")  # 2MB
            wds = []
            for c in range(NCHUNK):
                wg_sb = load_w(W["w_gate"], cols=(1024 * c, 1024 * (c + 1)), name=f"wg{c}",
                               pool=pwb, dt=BF16)
                wu_sb = load_w(W["w_up"], cols=(1024 * c, 1024 * (c + 1)), name=f"wu{c}",
                               pool=pwb, dt=BF16)
                for mi in range(8):
                    gps = psA.tile([128, 512], F32, tag="psA", name=f"g{c}_{mi}")
                    for t in range(HT):
                        nc.tensor.matmul(gps[:, :R],
                                         wg_sb[:, t, 128 * mi:128 * (mi + 1)],
                                         hn2b[:, t, :],
                                         start=(t == 0), stop=(t == HT - 1))
                    ups = psA.tile([128, 512], F32, tag="psA", name=f"u{c}_{mi}")
                    for t in range(HT):
                        nc.tensor.matmul(ups[:, :R],
                                         wu_sb[:, t, 128 * mi:128 * (mi + 1)],
                                         hn2b[:, t, :],
                                         start=(t == 0), stop=(t == HT - 1))
                    gsil = p2.tile([128, R], BF16, tag="gsil", name=f"gs{c}_{mi}")
                    nc.scalar.activation(gsil[:], gps[:, :R], AF.Silu, scale=DS)
                    # act = (up * DS) * silu(gate * DS), bf16 out
                    nc.vector.scalar_tensor_tensor(
                        act_full[:, 8 * c + mi, :], ups[:, :R], DS, gsil[:],
                        ALU.mult, ALU.mult)
            for c in range(NCHUNK):
                wds.append(load_w(W["w_down"], rows=(1024 * c, 1024 * (c + 1)), name=f"wd{c}"))
            out_sb = p1.tile([128, HT, R], F32, tag="out_sb")
            for m in range(HT):
                dps = psB.tile([128, 512], F32, tag="psB", name=f"dp{m}")
                for s in range(I // 128):
                    wd = wds[s // 8]
                    nc.tensor.matmul(dps[:, :R],
                                     wd[:, s % 8, 128 * m:128 * (m + 1)],
                                     act_full[:, s, :],
                                     start=(s == 0), stop=(s == I // 128 - 1))
                nc.vector.scalar_tensor_tensor(
                    out_sb[:, m, :], dps[:, :R], DS, h2[:, m, :], ALU.mult, ALU.add)
            nc.sync.dma_start(outT.rearrange("(t p) q -> p t q", p=128), out_sb[:])

    _split_multi_waits(nc)
    _CACHED_MODULE = nc
    return nc


def prep_in_maps(hidden_states, context, sa_norm_w, sa_wq, sa_wk, sa_wv, sa_wo,
                 ca_norm_w, ca_wq, ca_wk, ca_wv, ca_wo,
                 mlp_norm_w, w_gate, w_up, w_down):
    f32 = np.float32
    x = np.asarray(hidden_states, f32).reshape(S, H)
    ctx = np.asarray(context, f32).reshape(S, H)
    xT_full = np.ascontiguousarray(x.T)                      # [H, S] f32
    ctxT_full = np.ascontiguousarray(ctx.T).astype(FP8NP)    # [H, S] fp8

    def f8(a):
        return np.ascontiguousarray(
            np.clip(np.asarray(a, f32) * WS, -240.0, 240.0)).astype(FP8NP)

    def bf(a):
        return np.ascontiguousarray(np.asarray(a, f32)).astype(BF16NP)

    sa_w = np.asarray(sa_norm_w, f32)[:, None]
    ca_w = np.asarray(ca_norm_w, f32)[:, None]
    mlp_w = np.asarray(mlp_norm_w, f32)[:, None]
    scale = HD ** -0.5
    shared = {
        "sa_wq": f8(sa_w * np.asarray(sa_wq, f32) * scale),
        "sa_wk": f8(sa_w * np.asarray(sa_wk, f32)),
        "sa_wv": f8(sa_w * np.asarray(sa_wv, f32)),
        "sa_wo": f8(sa_wo),
        "ca_wq": f8(ca_w * np.asarray(ca_wq, f32) * scale),
        "ca_wk": f8(ca_wk),
        "ca_wv": f8(ca_wv),
        "ca_wo": f8(ca_wo),
        "w_gate": bf(mlp_w * np.asarray(w_gate, f32) * WS),
        "w_up": bf(mlp_w * np.asarray(w_up, f32) * WS),
        "w_down": f8(w_down),
    }
    in_maps = []
    for r in range(NC):
        m = dict(shared)
        m["xT"] = np.ascontiguousarray(xT_full[:, r * R:(r + 1) * R])
        m["ctxT"] = np.ascontiguousarray(ctxT_full[:, r * R:(r + 1) * R])
        in_maps.append(m)
    return in_maps


def run_spmd(in_maps, **kwargs):
    from concourse.bass_utils import run_bass_kernel_spmd
    nc = build_module()
    return run_bass_kernel_spmd(nc, in_maps, core_ids=list(range(NC)), **kwargs)


def kernel(**inputs):
    in_maps = prep_in_maps(**inputs)
    res = run_spmd(in_maps)
    out = np.empty((1, S, H), np.float32)
    for r in range(NC):
        out[0, r * R:(r + 1) * R, :] = res.results[r]["outT"].T
    return out
